# revision 16
# baseline (speedup 1.0000x reference)
"""Trainium2 Bass kernel for CrossAttention (self-attention) nn module.

Reference computation (B=2, N=4096, D=512, H=8, DH=64):
    q, k, v = x@Wq, x@Wk, x@Wv          # [B, N, 512]
    per head: S = q k^T / sqrt(64); P = softmax(S); O = P v
    out = concat_heads(O) @ Wo + bo     # [B, N, 512]

Sharding: batch*head-pair across 8 cores. Core c handles batch c//4 and
head pair c%4 (heads 2p, 2p+1). Each core computes its two heads'
attention plus its partial output projection; the host sums the four
partials per batch and adds the bias.

Device-side strategy (per core), v3 — fp16 projections/scores, fp8
softmax/PV with DoubleRow, denominators folded into PV:
  - QT/KT/VT [128(2 heads x 64), 4096] fp16 via PE matmuls.
  - S^T[keys, q] = K Q^T per head, K=64 row-packed: head0 in PE rows
    0-63 -> bank A of a [128, 1024] 2-bank PSUM tile, head1 in rows
    64-127 -> bank B (tile_position row tiling).
  - exp -> fp8e4 "pt", one engine per bank concurrently: ScalarE runs
    activation Exp on head0's bank; VectorE runs the fp8 Schraudolph bit
    trick on head1's bank: int8(S*c1 + c2) reinterpreted as fp8e4 IS
    approximately exp(S*scale) (one tensor_scalar instruction).
  - pt tiles are [128, 2(kb parity), 2(head), 512] fp8 per kb-PAIR: the
    DoubleRow interleave layout. PV contracts 256 keys per matmul via
    perf_mode=DoubleRow (fp8, 2 MACs/cell/cycle): lhsT = v8aug
    [128, 2, 65] = kb-pair interleaved V with a ones column at index 64,
    so PV row 64 accumulates the softmax denominator for free. DR can
    only target PSUM partitions 0-64, so each head accumulates in its
    own [65, 512] bank; PV for qc runs during qc+1's key loop (the two
    banks ring-1 through the chain below).
  - per qc epilogue (paced through qc+1's slots): copy each pso_h
    [65, 512] -> fp16 otn_h (row 64 = den); a tiny DMA transposes the
    two den rows to query-partition layout [128, 2, 4]; one DVE
    reciprocal gives dinvq. Output projection is per head (K=64,
    lhsT = UNNORMALIZED otn_h, rhs = Wo rows); normalization happens
    after the projection where 1/den is a per-PARTITION scalar:
    t1 = psY1 * dinvq1 (tensor_scalar), y = psY0 * dinvq0 + t1
    (scalar_tensor_tensor, fp16 out) -> DMA to DRAM (host upcasts,
    sums partials, adds bias).
"""

import math
import os
import sys

import numpy as np

for _p in ("/opt/trn_rl_repo", "/root/.axon_site/_ro/trn_rl_repo"):
    if os.path.isdir(_p) and _p not in sys.path:
        sys.path.insert(0, _p)

import concourse.bass as bass  # noqa: E402
import concourse.mybir as mybir  # noqa: E402
from concourse import bacc  # noqa: E402
from concourse.bass_utils import run_bass_kernel_spmd  # noqa: E402
from concourse.tile import TileContext  # noqa: E402

B, N, D = 2, 4096, 512
H, DH = 8, 64
P = 128                 # SBUF partitions / token block
KB = N // P             # 32 key blocks
NP = KB // 2            # 16 key-block pairs (DoubleRow contraction unit)
QC = N // 512            # 8 query column blocks of 512
KCH = D // P            # 4 contraction chunks for the projections
SCALE = DH ** -0.5
NCORES = 8

# Schraudolph exp in the fp8e4 bit domain: int8(x*C1 + C2).view(fp8e4)
# ~= exp(x * SCALE).  C1 = SCALE * 2^3 / ln 2; C2 = 7*2^3 - corr.
EXP_C1 = SCALE * 8.0 / math.log(2.0)
EXP_C2 = 56.0 - 0.3494

# knobs for test.py
TRACE = False
LAST_RESULT = None

_CACHED_NC = None


def build_nc():
    f32 = mybir.dt.float32
    f16 = mybir.dt.float16
    f8 = mybir.dt.float8e4
    i8 = mybir.dt.int8
    Exp = mybir.ActivationFunctionType.Exp
    Mult = mybir.AluOpType.mult
    Add = mybir.AluOpType.add
    DR = mybir.MatmulPerfMode.DoubleRow

    nc = bacc.Bacc()
    xT = nc.declare_dram_parameter("xT", [D, N], f16, isOutput=False)
    wq = nc.declare_dram_parameter("wq", [D, P], f16, isOutput=False)
    wk = nc.declare_dram_parameter("wk", [D, P], f16, isOutput=False)
    wv = nc.declare_dram_parameter("wv", [D, P], f16, isOutput=False)
    wop_d = nc.declare_dram_parameter("wop", [DH, 2, D], f16, isOutput=False)
    ident_d = nc.declare_dram_parameter("ident", [P, P], f16, isOutput=False)
    y = nc.declare_dram_parameter("y", [N, D], f16, isOutput=True)

    with TileContext(nc) as tc:
        with (
            tc.tile_pool(name="persist", bufs=1) as persist,
            tc.tile_pool(name="work", bufs=3) as work,
            tc.tile_pool(name="otnp", bufs=2) as otnp,
            tc.tile_pool(name="ysbp", bufs=2) as ysbp,
            tc.tile_pool(name="ptp", bufs=19) as ptp,
            tc.tile_pool(name="ps12", bufs=2, space="PSUM") as ps12,
            tc.tile_pool(name="ps_o", bufs=1, space="PSUM") as ps_op,
            tc.tile_pool(name="ps_y", bufs=1, space="PSUM") as ps_yp,
        ):
            # ---------------- prologue: loads ----------------
            # small weight DMAs first; xt in cc-major order so the first
            # projection column-pair's inputs land earliest
            xt_sb = persist.tile([P, KCH, N], f16, tag="xt")
            wq_sb = persist.tile([P, KCH, P], f16, tag="wq")
            wk_sb = persist.tile([P, KCH, P], f16, tag="wk")
            wv_sb = persist.tile([P, KCH, P], f16, tag="wv")
            for w_sb, w_d in ((wq_sb, wq), (wk_sb, wk), (wv_sb, wv)):
                nc.sync.dma_start(
                    out=w_sb, in_=w_d.rearrange("(c p) m -> p c m", p=P)
                )
            wop2 = persist.tile([DH, 2, D], f16, tag="wop")
            nc.sync.dma_start(out=wop2, in_=wop_d[:, :, :])
            ident = persist.tile([P, P], f16, tag="ident")
            nc.sync.dma_start(out=ident, in_=ident_d[:, :])

            qt = persist.tile([P, N], f16, tag="qt")
            kt = persist.tile([P, N], f16, tag="kt")
            vt = persist.tile([P, N], f16, tag="vt")
            # DoubleRow V: v8aug[p, pair, h, i, 0:64] = V[key=pair*256+i*128+p,
            # h*64+d]; col 64 = 1.0 (denominator fold), 65-79 pad for the
            # 16B interleave-stride rule.
            v8aug = persist.tile([P, NP, 2, 2, 80], f8, tag="vaug")
            for h in range(2):
                nc.vector.memset(v8aug[:, :, h, :, 64:65], 1.0)
            ndma = 0
            for cc in range(4):
                csl = slice(cc * 1024, (cc + 1) * 1024)
                for c in range(KCH):
                    eng = nc.scalar if (cc == 0 or ndma % 2 == 1) else nc.sync
                    eng.dma_start(
                        out=xt_sb[:, c, csl], in_=xT[c * P:(c + 1) * P, csl]
                    )
                    ndma += 1

            # ---------------- projections ----------------
            ncopy = [0]

            def cast_copy(dst, src):
                if ncopy[0] % 2 == 0:
                    nc.vector.tensor_copy(dst, src)
                else:
                    nc.scalar.copy(dst, src)
                ncopy[0] += 1

            def emit_col2(dst, w_sb, colpair, pool):
                """Two 512-col projection blocks through one 2-bank psum."""
                psp = pool.tile([P, 1024], f32, tag=pool._ctag, name="psp")
                for j in range(2):
                    col = colpair * 2 + j
                    csl = slice(col * 512, (col + 1) * 512)
                    for c in range(KCH):
                        nc.tensor.matmul(
                            psp[:, j * 512:(j + 1) * 512],
                            lhsT=w_sb[:, c, :],
                            rhs=xt_sb[:, c, csl],
                            start=(c == 0),
                            stop=(c == KCH - 1),
                        )
                csl2 = slice(colpair * 1024, (colpair + 1) * 1024)
                cast_copy(dst[:, csl2], psp)

            def emit_col(dst, w_sb, col, pool):
                psp = pool.tile([P, 512], f32, tag=pool._ctag, name="psp")
                csl = slice(col * 512, (col + 1) * 512)
                for c in range(KCH):
                    nc.tensor.matmul(
                        psp,
                        lhsT=w_sb[:, c, :],
                        rhs=xt_sb[:, c, csl],
                        start=(c == 0),
                        stop=(c == KCH - 1),
                    )
                cast_copy(dst[:, csl], psp)

            ps12._ctag = "ps12"
            ps_yp._ctag = "psY"
            # kt (all keys), qt's first query block, and V/v8aug are needed
            # before/within qc 0; the other qt columns stream in during
            # qc 0 (emit_prologue_tail below) through the idle psY bank.
            for colpair in range(4):
                emit_col2(kt, wk_sb, colpair, ps12)
            emit_col(qt, wq_sb, 0, ps_yp)
            for colpair in range(4):
                emit_col2(vt, wv_sb, colpair, ps12)

            def emit_vaug(batch):
                """Transpose 4 key blocks of VT and store in DoubleRow
                interleave layout (2 kb pairs per batch)."""
                psT4 = ps12.tile([P, 4, P], f16, tag="ps12", name="psT4")
                for t in range(4):
                    kb = batch * 4 + t
                    nc.tensor.transpose(
                        psT4[:, t, :], vt[:, kb * P:(kb + 1) * P], ident
                    )
                # psT4 dims: [p, (pairD i), (h d)] -> v8aug [p, pairD, h, i, d]
                # (engine APs allow at most 3 free dims, so split per head)
                src = psT4.rearrange("p (a i) (h d) -> p a h i d", a=2, h=2)
                for h in range(2):
                    cast_copy(
                        v8aug[:, batch * 2:batch * 2 + 2, h, :, 0:DH],
                        src[:, :, h, :, :],
                    )

            for batch in range(8):
                emit_vaug(batch)

            def emit_prologue_tail(kb):
                # qt columns 1-7 stream in during qc 0 through the psY bank
                if kb % 4 == 0 and 4 <= kb <= 28:
                    emit_col(qt, wq_sb, kb // 4, ps_yp)

            # ---------------- attention + output projection ----------------
            state = {}

            def emit_step(qc, kb):
                """S^T for both heads (row-packed, the two banks of one
                [128,1024] PSUM tile) + exp -> fp8 pt, one bank per engine:
                ScalarE activation-Exp on head0's bank, VectorE fp8
                Schraudolph on head1's bank (concurrent engines)."""
                pair, i = kb // 2, kb % 2
                ksl = slice(kb * P, (kb + 1) * P)
                qsl = slice(qc * 512, (qc + 1) * 512)
                ps = ps12.tile([P, 1024], f32, tag="ps12", name="ps")
                nc.tensor.matmul(ps[:, 0:512], lhsT=kt[0:DH, ksl], rhs=qt[0:DH, qsl])
                nc.tensor.matmul(ps[:, 512:1024], lhsT=kt[DH:P, ksl], rhs=qt[DH:P, qsl])
                if i == 0:
                    state[(qc, pair)] = ptp.tile([P, 2, 2, 512], f8, tag="pt", name="pt")
                pt = state[(qc, pair)]
                nc.scalar.activation(
                    pt[:, i, 0, :], ps[:, 0:512], func=Exp, scale=SCALE
                )
                nc.vector.tensor_scalar(
                    out=pt[:, i, 1, :].bitcast(i8),
                    in0=ps[:, 512:1024],
                    scalar1=float(EXP_C1),
                    scalar2=float(EXP_C2),
                    op0=Mult,
                    op1=Add,
                )

            def emit_pv(qc, pair):
                """DoubleRow PV: 256-key contraction per matmul, M=65 (the
                ones column accumulates the denominator in row 64). DR can
                only hit PSUM partitions 0-64, so one bank per head."""
                pt = state[(qc, pair)]
                if pair == 0:
                    state[(qc, "o0")] = ps_op.tile([DH + 1, 512], f32, tag="psO0", name="pso0")
                    state[(qc, "o1")] = ps_op.tile([DH + 1, 512], f32, tag="psO1", name="pso1")
                for h in range(2):
                    nc.tensor.matmul(
                        state[(qc, "o%d" % h)],
                        lhsT=v8aug[:, pair, h, :, 0:DH + 1],
                        rhs=pt[:, :, h, :],
                        start=(pair == 0),
                        stop=(pair == NP - 1),
                        perf_mode=DR,
                        skip_group_check=True,
                    )
                state.pop((qc, pair))

            def emit_otn(qc):
                # evacuate both PV banks to fp16 (row 64 = denominator)
                for h in range(2):
                    otn = otnp.tile([DH + 1, 512], f16, tag="otn%d" % h, name="otn")
                    cast_copy(otn, state.pop((qc, "o%d" % h)))
                    state[(qc, "otn%d" % h)] = otn

            def emit_dendma(qc):
                # transpose the two fp16 den rows into query-partition
                # layout: dq16[p, h, s] = den_h[s*128 + p]
                dq16 = work.tile([P, 2, 4], f16, tag="dq", name="dq16")
                for h in range(2):
                    den_row = state[(qc, "otn%d" % h)][DH:DH + 1, :]
                    for s in range(4):
                        nc.sync.dma_start(
                            out=dq16[:, h, s:s + 1],
                            in_=den_row[:, s * P:(s + 1) * P].rearrange(
                                "o (p u) -> o p u", u=1
                            ),
                        )
                state[(qc, "dq")] = dq16

            def emit_recip(qc):
                dq16 = state.pop((qc, "dq"))
                dq32 = work.tile([P, 8], f32, tag="dq32", name="dq32")
                nc.vector.tensor_copy(dq32, dq16.rearrange("p h s -> p (h s)"))
                dinvq = work.tile([P, 2, 4], f32, tag="dinvq", name="dinvq")
                nc.vector.reciprocal_approx_fast(
                    out=dinvq.rearrange("p h s -> p (h s)"), in_=dq32
                )
                state[(qc, "dinvq")] = dinvq

            def emit_proj1(qc, sub):
                # head1 projection (unnormalized) + scale to SBUF
                ssl = slice(sub * P, (sub + 1) * P)
                psY = ps_yp.tile([P, 512], f32, tag="psY", name="psY")
                nc.tensor.matmul(
                    psY, lhsT=state[(qc, "otn1")][0:DH, ssl], rhs=wop2[:, 1, :]
                )
                t1 = work.tile([P, 512], f16, tag="t1", name="t1")
                nc.vector.tensor_scalar(
                    out=t1,
                    in0=psY,
                    scalar1=state[(qc, "dinvq")][:, 1, sub:sub + 1],
                    scalar2=None,
                    op0=Mult,
                )
                state[(qc, "t1")] = t1

            def emit_proj0(qc, sub):
                # head0 projection + fused normalize-and-add -> fp16 y tile
                ssl = slice(sub * P, (sub + 1) * P)
                psY = ps_yp.tile([P, 512], f32, tag="psY", name="psY")
                nc.tensor.matmul(
                    psY, lhsT=state[(qc, "otn0")][0:DH, ssl], rhs=wop2[:, 0, :]
                )
                ysb = ysbp.tile([P, 512], f16, tag="ysb", name="ysb")
                nc.vector.scalar_tensor_tensor(
                    out=ysb,
                    in0=psY,
                    scalar=state[(qc, "dinvq")][:, 0, sub:sub + 1],
                    in1=state.pop((qc, "t1")),
                    op0=Mult,
                    op1=Add,
                )
                r0 = qc * 512 + sub * P
                nc.sync.dma_start(out=y[r0:r0 + P, :], in_=ysb)
                if sub == 3:
                    state.pop((qc, "otn0"))
                    state.pop((qc, "otn1"))
                    state.pop((qc, "dinvq"))

            def emit_tail(qc, kb):
                """Epilogue work for qc, paced across the kb slots of qc+1:
                kb 0-15 run qc's 16 PV pairs (pso banks are ring-1, freed
                at kb 16), then the den chain and per-head projections."""
                if kb <= 15:
                    emit_pv(qc, kb)
                elif kb == 16:
                    emit_otn(qc)
                elif kb == 17:
                    emit_dendma(qc)
                elif kb == 18:
                    emit_recip(qc)
                elif kb in (19, 21, 23, 25):
                    emit_proj1(qc, (kb - 19) // 2)
                elif kb in (20, 22, 24, 26):
                    emit_proj0(qc, (kb - 20) // 2)

            for qc in range(QC):
                for kb in range(KB):
                    emit_step(qc, kb)
                    if qc == 0:
                        emit_prologue_tail(kb)
                    else:
                        emit_tail(qc - 1, kb)

            def emit_dummy():
                # negligible-work matmul (K=1, M=1, N=512) that keeps the
                # PE duty cycle up so HAM holds K=8/8 through the tail
                scr = ps12.tile([1, 512], f32, tag="ps12", name="scr")
                nc.tensor.matmul(scr, lhsT=wop2[0:1, 0, 0:1], rhs=vt[0:1, 0:512])

            for kb in range(28):
                emit_tail(QC - 1, kb)
                if 16 <= kb <= 18:
                    # the den chain (copies/DMA/recip) has no PE work; keep
                    # the PE busy so HAM never sees an idle window
                    for _ in range(5):
                        emit_dummy()
                elif 19 <= kb <= 26:
                    emit_dummy()
                    emit_dummy()

    if not nc.is_finalized():
        nc.finalize()
    return nc


def _get_nc():
    global _CACHED_NC
    if _CACHED_NC is None:
        _CACHED_NC = build_nc()
    return _CACHED_NC


def make_in_maps(x, Wq, Wk, Wv, Wo):
    f16 = np.float16
    in_maps = []
    for c in range(NCORES):
        b, p = c // 4, c % 4
        cols = slice(p * P, (p + 1) * P)
        wop2 = np.ascontiguousarray(
            Wo[cols, :].reshape(2, DH, D).transpose(1, 0, 2)
        )
        in_maps.append({
            "xT": np.ascontiguousarray(x[b].T).astype(f16),
            "wq": np.ascontiguousarray(Wq[:, cols]).astype(f16),
            "wk": np.ascontiguousarray(Wk[:, cols]).astype(f16),
            "wv": np.ascontiguousarray(Wv[:, cols]).astype(f16),
            "wop": wop2.astype(f16),
            "ident": np.eye(P, dtype=f16),
        })
    return in_maps


def kernel(x, Wq, Wk, Wv, Wo, bo):
    global LAST_RESULT
    x = np.asarray(x, dtype=np.float32)
    Wq = np.asarray(Wq, dtype=np.float32)
    Wk = np.asarray(Wk, dtype=np.float32)
    Wv = np.asarray(Wv, dtype=np.float32)
    Wo = np.asarray(Wo, dtype=np.float32)
    bo = np.asarray(bo, dtype=np.float32)

    in_maps = make_in_maps(x, Wq, Wk, Wv, Wo)
    nc = _get_nc()
    res = run_bass_kernel_spmd(nc, in_maps, list(range(NCORES)), trace=TRACE)
    LAST_RESULT = res

    out = np.zeros((B, N, D), dtype=np.float32)
    for c in range(NCORES):
        out[c // 4] += res.results[c]["y"].astype(np.float32)
    out += bo[None, None, :]
    return out
